# revision 5
# baseline (speedup 1.0000x reference)
"""Trainium2 Bass kernel for linear attention over external memory.

Computes out = x @ (keys^T @ vals) for
  x [4, 2048, 1024] f32, keys/vals [65536, 1024] f32.

Sharding across 8 NeuronCores: keys/vals sharded along the memory dim M
(8192 rows per core); x sharded by token (1024 rows per core).

v2 structure (vs v1): the kv reduction is split into two partial
AllReduces so the collective latency hides under compute.
  - x loads + PE transposes run first, filling the DMA ramp.
  - Phase A (chunks 0..39): grouped PSUM accumulation drained to an
    SBUF f32 accumulator; cast to bf16 and AllReduced (2MB) while
    phase B computes.
  - Phase B (chunks 40..63): one PSUM accumulation group per column
    half; the h0 half's AllReduce (1MB) hides under the h1 pass
    compute (keys are re-read from HBM for the second pass).
  - Stage 4 (out = x @ kv) consumes each half as soon as its two
    partial-AR results are summed.
"""

import numpy as np

# Problem shapes (hardcoded per contract).
B, S, D = 4, 2048, 1024
M = 65536
NCORES = 8
P = 128
T = (B * S) // NCORES          # 1024 tokens per core
KM = M // NCORES               # 8192 memory rows per core
NC_ = KM // P                  # 64 k-chunks
G = 8                          # chunks per phase-A PSUM group
NGA = 5                        # phase-A groups (40 chunks)
NCA = NGA * G                  # 40
NCB = NC_ - NCA                # 24 phase-B chunks
DB = D // P                    # 8 d-blocks
HALF = D // 2                  # 512
TCH = T // P                   # 8 token chunks

_CACHE = {}


def _build_nc():
    import concourse.bacc as bacc
    import concourse.tile as tile
    from concourse import mybir
    from concourse.masks import make_identity

    f32 = mybir.dt.float32
    f32r = mybir.dt.float32r
    bf16 = mybir.dt.bfloat16
    ACT_COPY = mybir.ActivationFunctionType.Copy

    nc = bacc.Bacc("TRN2", target_bir_lowering=False, debug=False,
                   num_devices=NCORES)

    xs_d = nc.dram_tensor("xs", [T, D], f32, kind="ExternalInput")
    ks_d = nc.dram_tensor("ks", [KM, D], f32r, kind="ExternalInput")
    vs_d = nc.dram_tensor("vs", [KM, D], f32r, kind="ExternalInput")
    out_d = nc.dram_tensor("out", [T, D], f32, kind="ExternalOutput")

    ks_r = ks_d.ap().rearrange("(c p) n -> c p n", p=P)   # [64, 128, 1024]
    vs_r = vs_d.ap().rearrange("(c p) n -> c p n", p=P)
    xs_r = xs_d.ap().rearrange("(c p) n -> c p n", p=P)   # [8, 128, 1024]

    with tile.TileContext(nc) as tc:
        with (
            tc.tile_pool(name="const", bufs=1) as const,
            tc.tile_pool(name="kfp", bufs=10) as kfp,
            tc.tile_pool(name="vfp", bufs=10) as vfp,
            tc.tile_pool(name="vhp", bufs=8) as vhp,
            tc.tile_pool(name="accp", bufs=2 * DB) as accp,
            tc.tile_pool(name="xstage", bufs=4) as xstage,
            tc.tile_pool(name="xtp", bufs=DB) as xtp,
            tc.tile_pool(name="kvev", bufs=2) as kvev,
            tc.tile_pool(name="kvfin", bufs=2) as kvfin,
            tc.tile_pool(name="outp", bufs=3) as outp,
            tc.tile_pool(name="ps", bufs=8, space="PSUM") as ps,
            tc.tile_pool(name="dram", bufs=8, space="DRAM") as dram,
        ):
            ident = const.tile([P, P], f32)
            make_identity(nc, ident)

            # Warm-up collective: arms the ncfw collective stream so the
            # first real AllReduce trigger doesn't pay the ~11us wake-up.
            warm = const.tile([P, 16], bf16)
            nc.gpsimd.memset(warm[:], 0.0)
            warm_in = dram.tile([P, 16], bf16, name="warm_in")
            warm_out = dram.tile([P, 16], bf16, name="warm_out",
                                 addr_space="Shared")
            nc.gpsimd.dma_start(out=warm_in[:], in_=warm[:])
            nc.gpsimd.collective_compute(
                "AllReduce",
                mybir.AluOpType.add,
                replica_groups=[list(range(NCORES))],
                ins=[warm_in.opt()],
                outs=[warm_out.opt()],
            )

            # ---- x loads (scalar+vector queues) + PE transposes ----
            # These fill the PE while the k/v DMA stream ramps up.
            xT = [xtp.tile([P, T], bf16, name=f"xT{j}", tag="xT")
                  for j in range(DB)]
            xf_tiles = []
            for i in range(TCH):
                xf = xstage.tile([P, D], f32, name="xf", tag="xf")
                eng = nc.scalar if i % 2 == 0 else nc.gpsimd
                eng.dma_start(out=xf[:], in_=xs_r[i])
                xf_tiles.append(xf)
            for i in range(TCH):
                xf = xf_tiles[i]
                for j in range(DB):
                    pst = ps.tile([P, P], f32, name="pst", tag="ps")
                    nc.tensor.transpose(
                        pst[:], xf[:, j * P:(j + 1) * P], ident[:])
                    nc.vector.tensor_copy(
                        out=xT[j][:, i * P:(i + 1) * P], in_=pst[:])

            # kv accumulator (phase A): tile (h*DB+j) holds
            # kv[j*128:(j+1)*128, h*512:(h+1)*512] as [128, 512] f32.
            acc = [accp.tile([P, HALF], f32, name=f"acc{i}", tag="acc")
                   for i in range(2 * DB)]

            # ---- phase A: chunks 0..NCA, grouped PSUM accumulation ----
            for g in range(NGA):
                kf = []
                vf = []
                for c in range(G):
                    kt = kfp.tile([P, D], f32r, name="kt", tag="kt")
                    vt = vfp.tile([P, D], f32r, name="vt", tag="vt")
                    nc.sync.dma_start(out=kt[:], in_=ks_r[g * G + c])
                    nc.sync.dma_start(out=vt[:], in_=vs_r[g * G + c])
                    kf.append(kt)
                    vf.append(vt)
                for h in range(2):
                    e0 = h * HALF
                    pst = [ps.tile([P, HALF], f32, name=f"kv{h}_{j}",
                                   tag="ps") for j in range(DB)]
                    for c in range(G):
                        for j in range(DB):
                            nc.tensor.matmul(
                                pst[j][:],
                                kf[c][:, j * P:(j + 1) * P],
                                vf[c][:, e0:e0 + HALF],
                                start=(c == 0), stop=(c == G - 1))
                    for j in range(DB):
                        if g == 0:
                            nc.vector.tensor_copy(
                                out=acc[h * DB + j][:], in_=pst[j][:])
                        else:
                            nc.vector.tensor_tensor(
                                out=acc[h * DB + j][:],
                                in0=pst[j][:],
                                in1=acc[h * DB + j][:],
                                op=mybir.AluOpType.add)

            # ---- AllReduce A: full 2MB kv partial (hidden by phase B) --
            kvevA = [kvev.tile([P, DB * HALF], bf16, name=f"kvevA{h}",
                               tag="kvev") for h in range(2)]
            binA = dram.tile([P, 2 * DB * HALF], bf16, name="binA")
            boutA = dram.tile([P, 2 * DB * HALF], bf16, name="boutA",
                              addr_space="Shared")
            for h in range(2):
                for j in range(DB):
                    sl = slice(j * HALF, (j + 1) * HALF)
                    nc.scalar.activation(
                        kvevA[h][:, sl], acc[h * DB + j][:], ACT_COPY)
                    nc.gpsimd.dma_start(
                        out=binA[:, h * DB * HALF + j * HALF:
                                 h * DB * HALF + (j + 1) * HALF],
                        in_=kvevA[h][:, sl])
            nc.gpsimd.collective_compute(
                "AllReduce",
                mybir.AluOpType.add,
                replica_groups=[list(range(NCORES))],
                ins=[binA.opt()],
                outs=[boutA.opt()],
            )
            # Readback of the A result (vector queue, off the gpsimd
            # bounce path). kvf[h] later accumulates the B result.
            kvf = [kvfin.tile([P, DB * HALF], bf16, name=f"kvf{h}",
                              tag="kvfin") for h in range(2)]
            for h in range(2):
                nc.scalar.dma_start(
                    out=kvf[h][:],
                    in_=boutA[:, h * DB * HALF:(h + 1) * DB * HALF])

            # ---- phase B: chunks NCA..64, one PSUM group per half ----
            # Keys are re-read for the second half pass so each pass
            # streams k (4KB lines) + v-half (2KB lines).
            binB = [dram.tile([P, DB * HALF], bf16, name=f"binB{h}")
                    for h in range(2)]
            boutB = [dram.tile([P, DB * HALF], bf16, name=f"boutB{h}",
                               addr_space="Shared") for h in range(2)]
            kvevB = []
            for h in range(2):
                e0 = h * HALF
                psB = [ps.tile([P, HALF], f32, name=f"kvB{h}_{j}",
                               tag="ps") for j in range(DB)]
                for c in range(NCB):
                    kt = kfp.tile([P, D], f32r, name="ktB", tag="kt")
                    vh = vhp.tile([P, HALF], f32r, name="vhB", tag="vh")
                    nc.sync.dma_start(out=kt[:], in_=ks_r[NCA + c])
                    nc.sync.dma_start(out=vh[:],
                                      in_=vs_r[NCA + c][:, e0:e0 + HALF])
                    for j in range(DB):
                        nc.tensor.matmul(
                            psB[j][:],
                            kt[:, j * P:(j + 1) * P],
                            vh[:],
                            start=(c == 0), stop=(c == NCB - 1))
                kvevBh = kvev.tile([P, DB * HALF], bf16, name=f"kvevB{h}",
                                   tag="kvev")
                kvevB.append(kvevBh)
                for j in range(DB):
                    sl = slice(j * HALF, (j + 1) * HALF)
                    nc.scalar.activation(kvevBh[:, sl], psB[j][:], ACT_COPY)
                    nc.gpsimd.dma_start(out=binB[h][:, sl],
                                        in_=kvevBh[:, sl])
                nc.gpsimd.collective_compute(
                    "AllReduce",
                    mybir.AluOpType.add,
                    replica_groups=[list(range(NCORES))],
                    ins=[binB[h].opt()],
                    outs=[boutB[h].opt()],
                )

            # ---- stage 4: out = x @ (kvA + kvB), per column half ----
            for h in range(2):
                kvBr = kvev.tile([P, DB * HALF], bf16, name=f"kvBr{h}",
                                 tag="kvev")
                nc.scalar.dma_start(out=kvBr[:], in_=boutB[h][:])
                nc.vector.tensor_tensor(
                    out=kvf[h][:], in0=kvBr[:], in1=kvf[h][:],
                    op=mybir.AluOpType.add)
                for i in range(TCH):
                    po = ps.tile([P, HALF], f32, name="po", tag="ps")
                    for j in range(DB):
                        nc.tensor.matmul(
                            po[:],
                            xT[j][:, i * P:(i + 1) * P],
                            kvf[h][:, j * HALF:(j + 1) * HALF],
                            start=(j == 0), stop=(j == DB - 1))
                    ob = outp.tile([P, HALF], f32, name="ob", tag="ob")
                    nc.scalar.activation(ob[:], po[:], ACT_COPY)
                    nc.scalar.dma_start(
                        out=out_d.ap()[i * P:(i + 1) * P,
                                       h * HALF:(h + 1) * HALF],
                        in_=ob[:])

    nc.compile()
    return nc


def _get_nc():
    if "nc" not in _CACHE:
        _CACHE["nc"] = _build_nc()
    return _CACHE["nc"]


def kernel(**inputs):
    from concourse.bass_utils import run_bass_kernel_spmd

    x = np.ascontiguousarray(np.asarray(inputs["x"], dtype=np.float32))
    keys = np.ascontiguousarray(np.asarray(inputs["keys"], dtype=np.float32))
    vals = np.ascontiguousarray(np.asarray(inputs["vals"], dtype=np.float32))
    xf = x.reshape(B * S, D)

    nc = _get_nc()
    in_maps = []
    for c in range(NCORES):
        in_maps.append({
            "xs": xf[c * T:(c + 1) * T],
            "ks": keys[c * KM:(c + 1) * KM],
            "vs": vals[c * KM:(c + 1) * KM],
        })
    res = run_bass_kernel_spmd(nc, in_maps, list(range(NCORES)))
    out = np.concatenate([res.results[c]["out"] for c in range(NCORES)],
                         axis=0)
    return out.reshape(B, S, D).astype(np.float32)


# revision 6
# speedup vs baseline: 1.1139x; 1.1139x over previous
"""Trainium2 Bass kernel for linear attention over external memory.

Computes out = x @ (keys^T @ vals) for
  x [4, 2048, 1024] f32, keys/vals [65536, 1024] f32.

Sharding across 8 NeuronCores: keys/vals sharded along the memory dim M
(8192 rows per core); x sharded by token (1024 rows per core).

v3 structure: the kv reduction is split into two partial AllReduces so
the collective latency hides under compute.
  - x loads (scalar queue) + half the PE transposes run first, filling
    the DMA ramp while the k/v stream starts.
  - Phase A (chunks 0..39): grouped PSUM accumulation drained to an
    SBUF f32 accumulator; cast to bf16 and AllReduced (2MB) while
    phase B computes.
  - Phase B (chunks 40..63): same structure; the last group orders
    drains j-outer so the h0 half's AllReduce (1MB) triggers ~17us
    before the h1 half's matmuls finish.
  - The remaining transposes fill the PE while AR_B0 completes; stage
    4 consumes each half as soon as its two partial results are summed.
"""

import numpy as np

# Problem shapes (hardcoded per contract).
B, S, D = 4, 2048, 1024
M = 65536
NCORES = 8
P = 128
T = (B * S) // NCORES          # 1024 tokens per core
KM = M // NCORES               # 8192 memory rows per core
NC_ = KM // P                  # 64 k-chunks
G = 8                          # chunks per PSUM group
NG = NC_ // G                  # 8 groups
NGA = 5                        # phase-A groups (40 chunks)
DB = D // P                    # 8 d-blocks
HALF = D // 2                  # 512
TCH = T // P                   # 8 token chunks
TFRONT = 4                     # token chunks transposed up front

_CACHE = {}


def _build_nc():
    import concourse.bacc as bacc
    import concourse.tile as tile
    from concourse import mybir
    from concourse.masks import make_identity

    f32 = mybir.dt.float32
    f32r = mybir.dt.float32r
    bf16 = mybir.dt.bfloat16
    ACT_COPY = mybir.ActivationFunctionType.Copy

    nc = bacc.Bacc("TRN2", target_bir_lowering=False, debug=False,
                   num_devices=NCORES)

    xs_d = nc.dram_tensor("xs", [T, D], f32, kind="ExternalInput")
    ks_d = nc.dram_tensor("ks", [KM, D], f32r, kind="ExternalInput")
    vs_d = nc.dram_tensor("vs", [KM, D], f32r, kind="ExternalInput")
    out_d = nc.dram_tensor("out", [T, D], f32, kind="ExternalOutput")

    ks_r = ks_d.ap().rearrange("(c p) n -> c p n", p=P)   # [64, 128, 1024]
    vs_r = vs_d.ap().rearrange("(c p) n -> c p n", p=P)
    xs_r = xs_d.ap().rearrange("(c p) n -> c p n", p=P)   # [8, 128, 1024]

    with tile.TileContext(nc) as tc:
        with (
            tc.tile_pool(name="const", bufs=1) as const,
            tc.tile_pool(name="kfp", bufs=10) as kfp,
            tc.tile_pool(name="vfp", bufs=10) as vfp,
            tc.tile_pool(name="accp", bufs=2 * DB) as accp,
            tc.tile_pool(name="xstage", bufs=4) as xstage,
            tc.tile_pool(name="xtp", bufs=DB) as xtp,
            tc.tile_pool(name="kvev", bufs=2) as kvev,
            tc.tile_pool(name="kvfin", bufs=2) as kvfin,
            tc.tile_pool(name="outp", bufs=3) as outp,
            tc.tile_pool(name="ps", bufs=8, space="PSUM") as ps,
            tc.tile_pool(name="dram", bufs=8, space="DRAM") as dram,
        ):
            ident = const.tile([P, P], f32)
            make_identity(nc, ident)

            # Warm-up collective: arms the ncfw collective stream so the
            # first real AllReduce trigger doesn't pay the wake-up.
            warm = const.tile([P, 16], bf16)
            nc.gpsimd.memset(warm[:], 0.0)
            warm_in = dram.tile([P, 16], bf16, name="warm_in")
            warm_out = dram.tile([P, 16], bf16, name="warm_out",
                                 addr_space="Shared")
            nc.gpsimd.dma_start(out=warm_in[:], in_=warm[:])
            nc.gpsimd.collective_compute(
                "AllReduce",
                mybir.AluOpType.add,
                replica_groups=[list(range(NCORES))],
                ins=[warm_in.opt()],
                outs=[warm_out.opt()],
            )

            # ---- x loads (scalar queue); front transposes fill ramp --
            xT = [xtp.tile([P, T], bf16, name=f"xT{j}", tag="xT")
                  for j in range(DB)]
            xf_tiles = []
            for i in range(TCH):
                xf = xstage.tile([P, D], f32, name="xf", tag="xf")
                nc.scalar.dma_start(out=xf[:], in_=xs_r[i])
                xf_tiles.append(xf)

            def transpose_chunk(i):
                xf = xf_tiles[i]
                for j in range(DB):
                    pst = ps.tile([P, P], f32, name="pst", tag="ps")
                    nc.tensor.transpose(
                        pst[:], xf[:, j * P:(j + 1) * P], ident[:])
                    nc.vector.tensor_copy(
                        out=xT[j][:, i * P:(i + 1) * P], in_=pst[:])

            for i in range(TFRONT):
                transpose_chunk(i)

            # kv accumulator: tile (h*DB+j) holds kv[j*128:(j+1)*128,
            # h*512:(h+1)*512] as [128, 512] f32.
            acc = [accp.tile([P, HALF], f32, name=f"acc{i}", tag="acc")
                   for i in range(2 * DB)]

            # ---- stage 2: grouped PSUM accumulation, two phases ----
            def stage2_group(g, first):
                kf = []
                vf = []
                for c in range(G):
                    kt = kfp.tile([P, D], f32r, name="kt", tag="kt")
                    vt = vfp.tile([P, D], f32r, name="vt", tag="vt")
                    nc.sync.dma_start(out=kt[:], in_=ks_r[g * G + c])
                    nc.sync.dma_start(out=vt[:], in_=vs_r[g * G + c])
                    kf.append(kt)
                    vf.append(vt)
                last = (g == NG - 1)
                for h in range(2):
                    e0 = h * HALF
                    pst = [ps.tile([P, HALF], f32, name=f"kv{h}_{j}",
                                   tag="ps") for j in range(DB)]
                    order = ([(j, c) for j in range(DB) for c in range(G)]
                             if last else
                             [(j, c) for c in range(G) for j in range(DB)])
                    for j, c in order:
                        nc.tensor.matmul(
                            pst[j][:],
                            kf[c][:, j * P:(j + 1) * P],
                            vf[c][:, e0:e0 + HALF],
                            start=(c == 0), stop=(c == G - 1))
                    for j in range(DB):
                        if first:
                            nc.vector.tensor_copy(
                                out=acc[h * DB + j][:], in_=pst[j][:])
                        else:
                            nc.vector.tensor_tensor(
                                out=acc[h * DB + j][:],
                                in0=pst[j][:],
                                in1=acc[h * DB + j][:],
                                op=mybir.AluOpType.add)

            # Phase A
            for g in range(NGA):
                stage2_group(g, first=(g == 0))

            # AllReduce A: full 2MB kv partial (hidden by phase B).
            kvevA = [kvev.tile([P, DB * HALF], bf16, name=f"kvevA{h}",
                               tag="kvev") for h in range(2)]
            binA = dram.tile([P, 2 * DB * HALF], bf16, name="binA")
            boutA = dram.tile([P, 2 * DB * HALF], bf16, name="boutA",
                              addr_space="Shared")
            for h in range(2):
                for j in range(DB):
                    sl = slice(j * HALF, (j + 1) * HALF)
                    nc.scalar.activation(
                        kvevA[h][:, sl], acc[h * DB + j][:], ACT_COPY)
                    nc.gpsimd.dma_start(
                        out=binA[:, h * DB * HALF + j * HALF:
                                 h * DB * HALF + (j + 1) * HALF],
                        in_=kvevA[h][:, sl])
            nc.gpsimd.collective_compute(
                "AllReduce",
                mybir.AluOpType.add,
                replica_groups=[list(range(NCORES))],
                ins=[binA.opt()],
                outs=[boutA.opt()],
            )
            # Readback of the A result (scalar queue). kvf[h] later
            # accumulates the B result.
            kvf = [kvfin.tile([P, DB * HALF], bf16, name=f"kvf{h}",
                              tag="kvfin") for h in range(2)]
            for h in range(2):
                nc.scalar.dma_start(
                    out=kvf[h][:],
                    in_=boutA[:, h * DB * HALF:(h + 1) * DB * HALF])

            # Phase B (acc is overwritten by the first group's drain)
            for g in range(NGA, NG):
                stage2_group(g, first=(g == NGA))

            # AllReduce B in two column halves; h0 triggers ~17us before
            # the h1 matmuls finish.
            binB = [dram.tile([P, DB * HALF], bf16, name=f"binB{h}")
                    for h in range(2)]
            boutB = [dram.tile([P, DB * HALF], bf16, name=f"boutB{h}",
                               addr_space="Shared") for h in range(2)]
            kvevB = []
            for h in range(2):
                kvevBh = kvev.tile([P, DB * HALF], bf16, name=f"kvevB{h}",
                                   tag="kvev")
                kvevB.append(kvevBh)
                for j in range(DB):
                    sl = slice(j * HALF, (j + 1) * HALF)
                    nc.scalar.activation(
                        kvevBh[:, sl], acc[h * DB + j][:], ACT_COPY)
                    nc.gpsimd.dma_start(out=binB[h][:, sl],
                                        in_=kvevBh[:, sl])
                nc.gpsimd.collective_compute(
                    "AllReduce",
                    mybir.AluOpType.add,
                    replica_groups=[list(range(NCORES))],
                    ins=[binB[h].opt()],
                    outs=[boutB[h].opt()],
                )

            # Remaining transposes fill the PE during the AR_B0 wait.
            for i in range(TFRONT, TCH):
                transpose_chunk(i)

            # ---- stage 4: out = x @ (kvA + kvB), per column half ----
            for h in range(2):
                kvBr = kvev.tile([P, DB * HALF], bf16, name=f"kvBr{h}",
                                 tag="kvev")
                nc.scalar.dma_start(out=kvBr[:], in_=boutB[h][:])
                nc.vector.tensor_tensor(
                    out=kvf[h][:], in0=kvBr[:], in1=kvf[h][:],
                    op=mybir.AluOpType.add)
                for i in range(TCH):
                    po = ps.tile([P, HALF], f32, name="po", tag="ps")
                    for j in range(DB):
                        nc.tensor.matmul(
                            po[:],
                            xT[j][:, i * P:(i + 1) * P],
                            kvf[h][:, j * HALF:(j + 1) * HALF],
                            start=(j == 0), stop=(j == DB - 1))
                    ob = outp.tile([P, HALF], f32, name="ob", tag="ob")
                    nc.scalar.activation(ob[:], po[:], ACT_COPY)
                    nc.scalar.dma_start(
                        out=out_d.ap()[i * P:(i + 1) * P,
                                       h * HALF:(h + 1) * HALF],
                        in_=ob[:])

    nc.compile()
    return nc


def _get_nc():
    if "nc" not in _CACHE:
        _CACHE["nc"] = _build_nc()
    return _CACHE["nc"]


def kernel(**inputs):
    from concourse.bass_utils import run_bass_kernel_spmd

    x = np.ascontiguousarray(np.asarray(inputs["x"], dtype=np.float32))
    keys = np.ascontiguousarray(np.asarray(inputs["keys"], dtype=np.float32))
    vals = np.ascontiguousarray(np.asarray(inputs["vals"], dtype=np.float32))
    xf = x.reshape(B * S, D)

    nc = _get_nc()
    in_maps = []
    for c in range(NCORES):
        in_maps.append({
            "xs": xf[c * T:(c + 1) * T],
            "ks": keys[c * KM:(c + 1) * KM],
            "vs": vals[c * KM:(c + 1) * KM],
        })
    res = run_bass_kernel_spmd(nc, in_maps, list(range(NCORES)))
    out = np.concatenate([res.results[c]["out"] for c in range(NCORES)],
                         axis=0)
    return out.reshape(B, S, D).astype(np.float32)
